# revision 6
# baseline (speedup 1.0000x reference)
"""Trainium2 Bass kernel: GQA attention decode step (B=16, S=4, H=32, KV=8, D=128,
4096-entry KV cache), tensor-parallel over the 8 KV heads across 8 NeuronCores.

Core i owns KV head i and query heads 4i..4i+3:
  - QKV projections computed on-device from the full hidden states (column-sharded
    weights), RoPE applied on-device with host-precomputed cos/sin tables.
  - Attention computed in "transposed score" layout: scoresT[key, (h,s)] =
    kT_chunk.T @ qT, softmax denominators via ones-vector matmuls (partition-dim
    reduction on the PE), V applied as the matmul stationary operand so the
    attention output lands directly in [d, (h,s)] layout for the o_proj.
  - o_proj is row-parallel; partial outputs are summed with an on-device
    ReduceScatter over the batch dimension (rank i receives batches 2i, 2i+1).

Host-side prep: shards/transposes weights and the K cache (K is shipped as
[B, D, keys] so no on-device transposes of the 33 MB cache are needed), and
repacks V into chunk-major layout for fully-contiguous DMA partition lines.
"""
import sys

sys.path.insert(0, "/opt/trn_rl_repo")

import numpy as np

import concourse.bass as bass  # noqa: E402
import concourse.mybir as mybir  # noqa: E402
import concourse.tile as tile  # noqa: E402
from concourse import bacc  # noqa: E402
from concourse.bass_utils import run_bass_kernel_spmd  # noqa: E402

F32 = mybir.dt.float32
F32R = mybir.dt.float32r
MULT = mybir.AluOpType.mult
SUB = mybir.AluOpType.subtract
ADD = mybir.AluOpType.add

B, S, HID = 16, 4, 4096
H, KV, D = 32, 8, 128
TC = 4092  # cache keys used (start_pos)
NTOK = B * S  # 64
NCORES = 8
HL = H // KV  # 4 query heads per core
HDL = HL * D  # 512
SCALE = float(D) ** -0.5
NFULL = TC // 128  # 31 full cache chunks
TAIL = TC - NFULL * 128  # 124


def build_graph():
    nc = bacc.Bacc(
        "TRN2",
        target_bir_lowering=False,
        debug=False,
        enable_asserts=True,
        num_devices=NCORES,
    )

    # ---- I/O ----
    xTp = nc.dram_tensor("xTp", [128, 32 * NTOK], F32, kind="ExternalInput").ap()
    q_wT = nc.dram_tensor("q_wT", [HID, HDL], F32, kind="ExternalInput").ap()
    q_b = nc.dram_tensor("q_b", [1, HDL], F32, kind="ExternalInput").ap()
    kv_wT = nc.dram_tensor("kv_wT", [HID, 2 * D], F32, kind="ExternalInput").ap()
    kv_b = nc.dram_tensor("kv_b", [1, 2 * D], F32, kind="ExternalInput").ap()
    o_wT = nc.dram_tensor("o_wT", [HDL, HID], F32, kind="ExternalInput").ap()
    kT_d = nc.dram_tensor("kT", [B, D, TC], F32, kind="ExternalInput").ap()
    v_d = nc.dram_tensor("v", [B, 128, NFULL * 128], F32, kind="ExternalInput").ap()
    vtail_d = nc.dram_tensor("vtail", [B, TAIL, 128], F32, kind="ExternalInput").ap()
    cos_d = nc.dram_tensor("cosN", [NTOK, D // 2], F32, kind="ExternalInput").ap()
    sin_d = nc.dram_tensor("sinN", [NTOK, D // 2], F32, kind="ExternalInput").ap()

    out_e = nc.dram_tensor("out", [NTOK // NCORES, HID], F32, kind="ExternalOutput").ap()
    knew_e = nc.dram_tensor("k_new", [NTOK, D], F32, kind="ExternalOutput").ap()
    vnew_e = nc.dram_tensor("v_new", [NTOK, D], F32, kind="ExternalOutput").ap()

    ident_d = nc.inline_tensor(np.eye(128, dtype=np.float32), name="ident128")
    ones_d = nc.inline_tensor(np.ones((128, 128), np.float32), name="ones128")

    Exp = mybir.ActivationFunctionType.Exp

    with tile.TileContext(nc) as tc:
        with (
            tc.tile_pool(name="const", bufs=1) as cp,
            tc.tile_pool(name="proj", bufs=1) as pp,
            tc.tile_pool(name="dram", bufs=1, space="DRAM") as dp,
        ):
            ident = cp.tile([128, 128], F32)
            nc.sync.dma_start(ident[:], ident_d.ap())
            ones = cp.tile([128, 128], F32)
            nc.sync.dma_start(ones[:], ones_d.ap())
            xt = cp.tile([128, 32 * NTOK], F32)  # [p, (c, t)]
            nc.sync.dma_start(xt[:], xTp[:])
            cosn = cp.tile([NTOK, D // 2], F32)
            nc.sync.dma_start(cosn[:], cos_d[:])
            sinn = cp.tile([NTOK, D // 2], F32)
            nc.sync.dma_start(sinn[:], sin_d[:])

            rs_in = dp.tile([NTOK, HID], F32)
            rs_out = dp.tile([NTOK // NCORES, HID], F32)

            # ================= Phase A: projections + RoPE =================
            with (
                tc.tile_pool(name="wA", bufs=3) as wp,
                tc.tile_pool(name="psA", bufs=1, space="PSUM") as psA,
                tc.tile_pool(name="psT", bufs=2, space="PSUM") as psT,
            ):
                q_ps = psA.tile([NTOK, HDL], F32)
                for c in range(32):
                    wt = wp.tile([128, HDL], F32, tag="qw")
                    nc.sync.dma_start(wt[:], q_wT[c * 128 : (c + 1) * 128, :])
                    nc.tensor.matmul(
                        q_ps[:],
                        (xt[:, c * NTOK : (c + 1) * NTOK]),
                        (wt[:]),
                        start=(c == 0),
                        stop=False,
                    )
                bq = wp.tile([1, HDL], F32, tag="qbias")
                nc.sync.dma_start(bq[:], q_b[:])
                nc.tensor.matmul(
                    q_ps[:], (ones[0:1, 0:NTOK]), (bq[:]), start=False, stop=True
                )

                kv_ps = psA.tile([NTOK, 2 * D], F32)
                for c in range(32):
                    wt = wp.tile([128, 2 * D], F32, tag="kvw")
                    nc.sync.dma_start(wt[:], kv_wT[c * 128 : (c + 1) * 128, :])
                    nc.tensor.matmul(
                        kv_ps[:],
                        (xt[:, c * NTOK : (c + 1) * NTOK]),
                        (wt[:]),
                        start=(c == 0),
                        stop=False,
                    )
                bkv = wp.tile([1, 2 * D], F32, tag="kvbias")
                nc.sync.dma_start(bkv[:], kv_b[:])
                nc.tensor.matmul(
                    kv_ps[:], (ones[0:1, 0:NTOK]), (bkv[:]), start=False, stop=True
                )

                q_nat = pp.tile([NTOK, HDL], F32)
                nc.scalar.copy(q_nat[:], q_ps[:])
                kv_nat = pp.tile([NTOK, 2 * D], F32)
                nc.scalar.copy(kv_nat[:], kv_ps[:])

                # v_new output (no rope)
                nc.sync.dma_start(vnew_e[:], kv_nat[:, D : 2 * D])

                # RoPE (natural layout [64, 128] per head): out1 = x1*cos - x2*sin,
                # out2 = x2*cos + x1*sin
                q_rope = pp.tile([NTOK, HDL], F32)
                k_rope = pp.tile([NTOK, D], F32)

                def rope(dst, src):
                    x1, x2 = src[:, 0:64], src[:, 64:128]
                    d1, d2 = dst[:, 0:64], dst[:, 64:128]
                    t1 = pp.tile([NTOK, 64], F32, tag="ropet1")
                    t2 = pp.tile([NTOK, 64], F32, tag="ropet2")
                    nc.vector.scalar_tensor_tensor(t1[:], x1, 1.0, cosn[:], MULT, MULT)
                    nc.vector.scalar_tensor_tensor(t2[:], x2, 1.0, sinn[:], MULT, MULT)
                    nc.vector.scalar_tensor_tensor(d1, t1[:], 1.0, t2[:], MULT, SUB)
                    t3 = pp.tile([NTOK, 64], F32, tag="ropet1")
                    t4 = pp.tile([NTOK, 64], F32, tag="ropet2")
                    nc.vector.scalar_tensor_tensor(t3[:], x2, 1.0, cosn[:], MULT, MULT)
                    nc.vector.scalar_tensor_tensor(t4[:], x1, 1.0, sinn[:], MULT, MULT)
                    nc.vector.scalar_tensor_tensor(d2, t3[:], 1.0, t4[:], MULT, ADD)

                for h in range(HL):
                    rope(q_rope[:, h * D : (h + 1) * D], q_nat[:, h * D : (h + 1) * D])
                rope(k_rope[:], kv_nat[:, 0:D])

                nc.sync.dma_start(knew_e[:], k_rope[:])

                # Transposes: q_ropedT [128, 4*64] (cols h*64 + t), kT_new [128, 64]
                qT_sb = pp.tile([128, HL * NTOK], F32)
                for h in range(HL):
                    tp = psT.tile([128, NTOK], F32, tag="tp")
                    nc.tensor.transpose(
                        tp[:], q_rope[:, h * D : (h + 1) * D], ident[0:NTOK, 0:NTOK]
                    )
                    nc.scalar.copy(qT_sb[:, h * NTOK : (h + 1) * NTOK], tp[:])
                kTn = pp.tile([128, NTOK], F32)
                tp = psT.tile([128, NTOK], F32, tag="tp")
                nc.tensor.transpose(tp[:], k_rope[:], ident[0:NTOK, 0:NTOK])
                nc.scalar.copy(kTn[:], tp[:])

                # per-batch v_new stationary tiles at partition base 0: [4, 16*128]
                vnb = pp.tile([S, B * D], F32)
                for b in range(B):
                    # DVE can't read at partition offsets that aren't 0/32/64/96;
                    # use an SBUF->SBUF DMA to gather each batch's 4 new-token v rows.
                    nc.sync.dma_start(
                        vnb[:, b * D : (b + 1) * D],
                        kv_nat[S * b : S * b + S, D : 2 * D],
                    )

                # new-token scoresT for all batches: [4 keys, (b, h, s)]
                qT_v = qT_sb[:].rearrange("p (h t) -> p h t", h=HL)
                scn_ps = psT.tile([S, B * HL * S], F32, tag="scn")
                for b in range(B):
                    nc.tensor.matmul(
                        scn_ps[:, b * 16 : (b + 1) * 16],
                        kTn[:, S * b : S * b + S],
                        qT_v[:, :, S * b : S * b + S],
                        start=True,
                        stop=True,
                    )
                expn = pp.tile([S, B * HL * S], F32)
                nc.scalar.activation(expn[:], scn_ps[:], Exp, scale=SCALE)

            # ================= Phase B: attention over the KV cache =================
            outT_all = pp.tile([128, HL * NTOK], F32)  # cols h*64 + b*4 + s
            with (
                tc.tile_pool(name="kv", bufs=2) as kvp,
                tc.tile_pool(name="exb", bufs=2) as exp_pool,
                tc.tile_pool(name="ps_sc", bufs=2, space="PSUM") as pssc,
                tc.tile_pool(name="ps_out", bufs=2, space="PSUM") as psout,
                tc.tile_pool(name="ps_sm", bufs=2, space="PSUM") as pssm,
                tc.tile_pool(name="ps_bc", bufs=1, space="PSUM") as psbc,
            ):
                for b in range(B):
                    kt = kvp.tile([128, TC], F32, tag="kt")
                    nc.sync.dma_start(kt[:], kT_d[b])
                    vt = kvp.tile([128, NFULL * 128], F32, tag="vt")
                    nc.sync.dma_start(vt[:], v_d[b])
                    vtl = kvp.tile([TAIL, 128], F32, tag="vtl")
                    nc.sync.dma_start(vtl[:], vtail_d[b])

                    qTb = qT_v[:, :, S * b : S * b + S]  # [128, 4, 4]

                    sc = pssc.tile([128, 512], F32, tag="sc")  # one PSUM bank
                    for c in range(NFULL):
                        nc.tensor.matmul(
                            sc[:, c * 16 : (c + 1) * 16],
                            kt[:, c * 128 : (c + 1) * 128],
                            qTb,
                            start=True,
                            stop=True,
                        )
                    nc.tensor.matmul(
                        sc[0:TAIL, NFULL * 16 : NFULL * 16 + 16],
                        kt[:, NFULL * 128 : TC],
                        qTb,
                        start=True,
                        stop=True,
                    )

                    ex = exp_pool.tile([128, 512], F32, tag="ex")
                    nc.scalar.activation(ex[:], sc[:], Exp, scale=SCALE)

                    outp = psout.tile([128, 16], F32, tag="outp")
                    for c in range(NFULL):
                        nc.tensor.matmul(
                            outp[:],
                            vt[:, c * 128 : (c + 1) * 128],
                            ex[:, c * 16 : (c + 1) * 16],
                            start=(c == 0),
                            stop=False,
                        )
                    nc.tensor.matmul(
                        outp[:],
                        vtl[:],
                        ex[0:TAIL, NFULL * 16 : NFULL * 16 + 16],
                        start=False,
                        stop=False,
                    )
                    nc.tensor.matmul(
                        outp[:],
                        vnb[:, b * D : (b + 1) * D],
                        expn[:, b * 16 : (b + 1) * 16],
                        start=False,
                        stop=True,
                    )

                    sm = pssm.tile([1, 16], F32, tag="sm")
                    for c in range(NFULL):
                        nc.tensor.matmul(
                            sm[:],
                            ones[:, 0:1],
                            ex[:, c * 16 : (c + 1) * 16],
                            start=(c == 0),
                            stop=False,
                        )
                    nc.tensor.matmul(
                        sm[:],
                        ones[0:TAIL, 0:1],
                        ex[0:TAIL, NFULL * 16 : NFULL * 16 + 16],
                        start=False,
                        stop=False,
                    )
                    nc.tensor.matmul(
                        sm[:],
                        ones[0:S, 0:1],
                        expn[:, b * 16 : (b + 1) * 16],
                        start=False,
                        stop=True,
                    )

                    rs = exp_pool.tile([1, 16], F32, tag="rs")
                    nc.vector.reciprocal(rs[:], sm[:])
                    # broadcast [1,16] across 128 partitions via ones outer product
                    bc = psbc.tile([128, 16], F32, tag="bc")
                    nc.tensor.matmul(bc[:], ones[0:1, :], rs[:], start=True, stop=True)
                    # DVE may read only one PSUM operand; stage the broadcast in SBUF
                    bcs = exp_pool.tile([128, 16], F32, tag="bcs")
                    nc.scalar.copy(bcs[:], bc[:])
                    # normalize and scatter into outT_all cols {h*64 + 4b + s}
                    dst = outT_all[:].rearrange("p (h bb t) -> p h bb t", h=HL, bb=B)[
                        :, :, b, :
                    ]
                    nc.vector.scalar_tensor_tensor(
                        dst,
                        outp[:].rearrange("p (h t) -> p h t", h=HL),
                        1.0,
                        bcs[:].rearrange("p (h t) -> p h t", h=HL),
                        MULT,
                        MULT,
                    )

            # ================= Phase C: o_proj partials =================
            with (
                tc.tile_pool(name="wC", bufs=3) as wc,
                tc.tile_pool(name="fin", bufs=2) as fp,
                tc.tile_pool(name="psC", bufs=2, space="PSUM") as psC,
            ):
                for n in range(HID // 512):
                    op_ps = psC.tile([NTOK, 512], F32, tag="op")
                    for h in range(HL):
                        wt = wc.tile([128, 512], F32, tag="ow")
                        nc.sync.dma_start(
                            wt[:], o_wT[h * 128 : (h + 1) * 128, n * 512 : (n + 1) * 512]
                        )
                        nc.tensor.matmul(
                            op_ps[:],
                            (outT_all[:, h * NTOK : (h + 1) * NTOK]),
                            (wt[:]),
                            start=(h == 0),
                            stop=(h == HL - 1),
                        )
                    fin = fp.tile([NTOK, 512], F32, tag="fin")
                    nc.scalar.copy(fin[:], op_ps[:])
                    nc.sync.dma_start(rs_in[:, n * 512 : (n + 1) * 512], fin[:])

            # ================= Phase D: ReduceScatter over batches =================
            nc.gpsimd.collective_compute(
                "ReduceScatter",
                mybir.AluOpType.add,
                ins=[rs_in.opt()],
                outs=[rs_out.opt()],
                replica_groups=[list(range(NCORES))],
            )
            nc.sync.dma_start(out_e[:], rs_out[:])

    nc.compile()
    return nc


_NC = None


def _get_nc():
    global _NC
    if _NC is None:
        _NC = build_graph()
    return _NC


def host_prep(
    hidden_states, positions, k_cache, v_cache, start_pos, q_w, q_b, k_w, k_b, v_w, v_b, o_w
):
    hs = np.ascontiguousarray(np.asarray(hidden_states, np.float32))
    pos = np.asarray(positions).reshape(NTOK)
    kc = np.asarray(k_cache, np.float32)
    vc = np.asarray(v_cache, np.float32)
    q_w = np.asarray(q_w, np.float32)
    q_b = np.asarray(q_b, np.float32)
    k_w = np.asarray(k_w, np.float32)
    k_b = np.asarray(k_b, np.float32)
    v_w = np.asarray(v_w, np.float32)
    v_b = np.asarray(v_b, np.float32)
    o_w = np.asarray(o_w, np.float32)
    assert int(start_pos) == TC, f"kernel compiled for start_pos={TC}"

    # [p, (c, t)] packing of hidden_states.T
    xTp = np.ascontiguousarray(
        hs.reshape(NTOK, 32, 128).transpose(2, 1, 0).reshape(128, 32 * NTOK)
    )
    half = D // 2
    inv_freq = (1.0 / (10000.0 ** (np.arange(0, half, dtype=np.float32) / half))).astype(
        np.float32
    )
    freqs = pos.astype(np.float32)[:, None] * inv_freq[None, :]
    cosN = np.cos(freqs).astype(np.float32)
    sinN = np.sin(freqs).astype(np.float32)

    maps = []
    for i in range(NCORES):
        kT = np.ascontiguousarray(kc[:, i, :TC, :].transpose(0, 2, 1))
        vs = vc[:, i, :TC, :]
        v_main = np.ascontiguousarray(
            vs[:, : NFULL * 128].reshape(B, NFULL, 128, 128).transpose(0, 2, 1, 3)
        ).reshape(B, 128, NFULL * 128)
        v_tail = np.ascontiguousarray(vs[:, NFULL * 128 :])
        maps.append(
            dict(
                xTp=xTp,
                q_wT=np.ascontiguousarray(q_w[i * HDL : (i + 1) * HDL, :].T),
                q_b=np.ascontiguousarray(q_b[i * HDL : (i + 1) * HDL].reshape(1, HDL)),
                kv_wT=np.ascontiguousarray(
                    np.concatenate(
                        [k_w[i * D : (i + 1) * D], v_w[i * D : (i + 1) * D]], 0
                    ).T
                ),
                kv_b=np.ascontiguousarray(
                    np.concatenate(
                        [k_b[i * D : (i + 1) * D], v_b[i * D : (i + 1) * D]]
                    ).reshape(1, 2 * D)
                ),
                o_wT=np.ascontiguousarray(o_w[:, i * HDL : (i + 1) * HDL].T),
                kT=kT,
                v=v_main,
                vtail=v_tail,
                cosN=cosN,
                sinN=sinN,
            )
        )
    return maps


def kernel(**inputs):
    nc = _get_nc()
    in_maps = host_prep(**inputs)
    res = run_bass_kernel_spmd(nc, in_maps, list(range(NCORES))).results
    out = np.concatenate([res[i]["out"] for i in range(NCORES)], 0).reshape(B, S, HID)
    k_new = np.stack(
        [res[i]["k_new"].reshape(B, S, D) for i in range(NCORES)], axis=1
    )
    v_new = np.stack(
        [res[i]["v_new"].reshape(B, S, D) for i in range(NCORES)], axis=1
    )
    return out, k_new, v_new


# revision 10
# speedup vs baseline: 1.7044x; 1.7044x over previous
"""Trainium2 Bass kernel: GQA attention decode step (B=16, S=4, H=32, KV=8, D=128,
4096-entry KV cache), tensor-parallel over the 8 KV heads across 8 NeuronCores.

Core i owns KV head i and query heads 4i..4i+3:
  - QKV projections computed on-device from the full hidden states (column-sharded
    weights), RoPE applied on-device with host-precomputed cos/sin tables.
  - Attention computed in "transposed score" layout: scoresT[key, (h,s)] =
    kT_chunk.T @ qT, softmax denominators via ones-vector matmuls (partition-dim
    reduction on the PE), V applied as the matmul stationary operand so the
    attention output lands directly in [d, (h,s)] layout for the o_proj.
  - o_proj is row-parallel; partial outputs are summed with an on-device
    ReduceScatter over the batch dimension (rank i receives batches 2i, 2i+1).

All TensorEngine-facing tensors are bf16 (fp32 matmuls cost 4 cycles/row and
double the instruction count; bf16 runs 1 cycle/row with fast weight loads).
HBM traffic stays fp32: everything is DMA'd as f32 and converted on-chip
(K + weights on ScalarE/VectorE, V on VectorE). PSUM accumulation is fp32
throughout, as are RoPE and the softmax reciprocals.

Host-side prep: shards/transposes weights; ships K as [B, D, 4096] (transposed,
zero-padded past start_pos) and V as [B, 128, 4096] chunk-major zero-padded, so
each per-batch DMA is 128 fully-contiguous 16 KB partition lines. The 4 zero
pad keys contribute exp(0)=1 to each softmax sum, compensated by subtracting
4.0 before the reciprocal.
"""
import sys

sys.path.insert(0, "/opt/trn_rl_repo")

import numpy as np

import concourse.bass as bass  # noqa: E402
import concourse.mybir as mybir  # noqa: E402
import concourse.tile as tile  # noqa: E402
from concourse import bacc  # noqa: E402
from concourse.bass_utils import run_bass_kernel_spmd  # noqa: E402

F32 = mybir.dt.float32
BF16 = mybir.dt.bfloat16
MULT = mybir.AluOpType.mult
SUB = mybir.AluOpType.subtract
ADD = mybir.AluOpType.add

B, S, HID = 16, 4, 4096
H, KV, D = 32, 8, 128
TC = 4092  # cache keys used (start_pos)
TP = 4096  # padded key count (4 zero keys -> exp(0)=1, compensated)
NPAD = TP - TC
NCH = TP // 128  # 32 key chunks per batch
NTOK = B * S  # 64
NCORES = 8
HL = H // KV  # 4 query heads per core
HDL = HL * D  # 512
SCALE = float(D) ** -0.5

Exp = mybir.ActivationFunctionType.Exp


def build_graph():
    nc = bacc.Bacc(
        "TRN2",
        target_bir_lowering=False,
        debug=False,
        enable_asserts=True,
        num_devices=NCORES,
    )

    # ---- I/O ----
    xTp = nc.dram_tensor("xTp", [128, 32 * NTOK], F32, kind="ExternalInput").ap()
    q_wT = nc.dram_tensor("q_wT", [HID, HDL], F32, kind="ExternalInput").ap()
    q_b = nc.dram_tensor("q_b", [1, HDL], F32, kind="ExternalInput").ap()
    kv_wT = nc.dram_tensor("kv_wT", [HID, 2 * D], F32, kind="ExternalInput").ap()
    kv_b = nc.dram_tensor("kv_b", [1, 2 * D], F32, kind="ExternalInput").ap()
    o_wT = nc.dram_tensor("o_wT", [HDL, HID], F32, kind="ExternalInput").ap()
    kT_d = nc.dram_tensor("kT", [B, D, TP], F32, kind="ExternalInput").ap()
    v_d = nc.dram_tensor("v", [B, 128, TP], F32, kind="ExternalInput").ap()
    cos_d = nc.dram_tensor("cosN", [NTOK, D // 2], F32, kind="ExternalInput").ap()
    sin_d = nc.dram_tensor("sinN", [NTOK, D // 2], F32, kind="ExternalInput").ap()

    out_e = nc.dram_tensor("out", [NTOK // NCORES, HID], F32, kind="ExternalOutput").ap()
    knew_e = nc.dram_tensor("k_new", [NTOK, D], F32, kind="ExternalOutput").ap()
    vnew_e = nc.dram_tensor("v_new", [NTOK, D], F32, kind="ExternalOutput").ap()

    ident_d = nc.inline_tensor(np.eye(128, dtype=np.float32), name="ident128")
    ones_d = nc.inline_tensor(np.ones((128, 128), np.float32), name="ones128")

    with tile.TileContext(nc) as tc:
        with (
            tc.tile_pool(name="const", bufs=1) as cp,
            tc.tile_pool(name="proj", bufs=1) as pp,
            tc.tile_pool(name="dram", bufs=1, space="DRAM") as dp,
        ):
            ident = cp.tile([128, 128], F32)
            nc.sync.dma_start(ident[:], ident_d.ap())
            ones = cp.tile([128, 128], F32)
            nc.sync.dma_start(ones[:], ones_d.ap())
            onesb = cp.tile([128, 128], BF16)
            nc.vector.tensor_copy(onesb[:], ones[:])
            xtf = cp.tile([128, 32 * NTOK], F32)  # [p, (c, t)]
            nc.sync.dma_start(xtf[:], xTp[:])
            xt = cp.tile([128, 32 * NTOK], BF16)
            nc.scalar.copy(xt[:], xtf[:])
            cosn = cp.tile([NTOK, D // 2], F32)
            nc.sync.dma_start(cosn[:], cos_d[:])
            sinn = cp.tile([NTOK, D // 2], F32)
            nc.sync.dma_start(sinn[:], sin_d[:])

            rs_in = dp.tile([NTOK, HID], F32)
            rs_out = dp.tile([NTOK // NCORES, HID], F32)

            # ================= Phase A: projections + RoPE =================
            with (
                tc.tile_pool(name="wA", bufs=3) as wp,
                tc.tile_pool(name="psA", bufs=1, space="PSUM") as psA,
                tc.tile_pool(name="psT", bufs=2, space="PSUM") as psT,
            ):
                q_ps = psA.tile([NTOK, HDL], F32)
                for c in range(32):
                    wtf = wp.tile([128, HDL], F32, tag="qwf")
                    nc.sync.dma_start(wtf[:], q_wT[c * 128 : (c + 1) * 128, :])
                    wt = wp.tile([128, HDL], BF16, tag="qw")
                    nc.vector.tensor_copy(wt[:], wtf[:])
                    nc.tensor.matmul(
                        q_ps[:],
                        xt[:, c * NTOK : (c + 1) * NTOK],
                        wt[:],
                        start=(c == 0),
                        stop=False,
                    )
                bqf = wp.tile([1, HDL], F32, tag="qbiasf")
                nc.sync.dma_start(bqf[:], q_b[:])
                bq = wp.tile([1, HDL], BF16, tag="qbias")
                nc.vector.tensor_copy(bq[:], bqf[:])
                nc.tensor.matmul(
                    q_ps[:], onesb[0:1, 0:NTOK], bq[:], start=False, stop=True
                )

                kv_ps = psA.tile([NTOK, 2 * D], F32)
                for c in range(32):
                    wtf = wp.tile([128, 2 * D], F32, tag="kvwf")
                    nc.sync.dma_start(wtf[:], kv_wT[c * 128 : (c + 1) * 128, :])
                    wt = wp.tile([128, 2 * D], BF16, tag="kvw")
                    nc.vector.tensor_copy(wt[:], wtf[:])
                    nc.tensor.matmul(
                        kv_ps[:],
                        xt[:, c * NTOK : (c + 1) * NTOK],
                        wt[:],
                        start=(c == 0),
                        stop=False,
                    )
                bkvf = wp.tile([1, 2 * D], F32, tag="kvbiasf")
                nc.sync.dma_start(bkvf[:], kv_b[:])
                bkv = wp.tile([1, 2 * D], BF16, tag="kvbias")
                nc.vector.tensor_copy(bkv[:], bkvf[:])
                nc.tensor.matmul(
                    kv_ps[:], onesb[0:1, 0:NTOK], bkv[:], start=False, stop=True
                )

                q_nat = pp.tile([NTOK, HDL], F32)
                nc.scalar.copy(q_nat[:], q_ps[:])
                kv_nat = pp.tile([NTOK, 2 * D], F32)
                nc.scalar.copy(kv_nat[:], kv_ps[:])

                # v_new output (no rope)
                nc.sync.dma_start(vnew_e[:], kv_nat[:, D : 2 * D])

                # RoPE (natural layout [64, 128] per head): out1 = x1*cos - x2*sin,
                # out2 = x2*cos + x1*sin
                q_rope = pp.tile([NTOK, HDL], F32)
                k_rope = pp.tile([NTOK, D], F32)

                def rope(dst, src):
                    x1, x2 = src[:, 0:64], src[:, 64:128]
                    d1, d2 = dst[:, 0:64], dst[:, 64:128]
                    t1 = pp.tile([NTOK, 64], F32, tag="ropet1")
                    t2 = pp.tile([NTOK, 64], F32, tag="ropet2")
                    nc.vector.scalar_tensor_tensor(t1[:], x1, 1.0, cosn[:], MULT, MULT)
                    nc.vector.scalar_tensor_tensor(t2[:], x2, 1.0, sinn[:], MULT, MULT)
                    nc.vector.scalar_tensor_tensor(d1, t1[:], 1.0, t2[:], MULT, SUB)
                    t3 = pp.tile([NTOK, 64], F32, tag="ropet1")
                    t4 = pp.tile([NTOK, 64], F32, tag="ropet2")
                    nc.vector.scalar_tensor_tensor(t3[:], x2, 1.0, cosn[:], MULT, MULT)
                    nc.vector.scalar_tensor_tensor(t4[:], x1, 1.0, sinn[:], MULT, MULT)
                    nc.vector.scalar_tensor_tensor(d2, t3[:], 1.0, t4[:], MULT, ADD)

                for h in range(HL):
                    rope(q_rope[:, h * D : (h + 1) * D], q_nat[:, h * D : (h + 1) * D])
                rope(k_rope[:], kv_nat[:, 0:D])

                nc.sync.dma_start(knew_e[:], k_rope[:])

                # Transposes: q_ropedT [128, 4*64] (cols h*64 + t), kT_new [128, 64]
                qT_sb = pp.tile([128, HL * NTOK], BF16)
                for h in range(HL):
                    tp = psT.tile([128, NTOK], F32, tag="tp")
                    nc.tensor.transpose(
                        tp[:], q_rope[:, h * D : (h + 1) * D], ident[0:NTOK, 0:NTOK]
                    )
                    nc.scalar.copy(qT_sb[:, h * NTOK : (h + 1) * NTOK], tp[:])
                kTn = pp.tile([128, NTOK], BF16)
                tp = psT.tile([128, NTOK], F32, tag="tp")
                nc.tensor.transpose(tp[:], k_rope[:], ident[0:NTOK, 0:NTOK])
                nc.scalar.copy(kTn[:], tp[:])

                # per-batch v_new stationary tiles at partition base 0 (DVE can't
                # read partition offsets like 4b, so gather rows via SBUF DMA)
                vnbf = pp.tile([S, B * D], F32)
                for b in range(B):
                    nc.sync.dma_start(
                        vnbf[:, b * D : (b + 1) * D],
                        kv_nat[S * b : S * b + S, D : 2 * D],
                    )
                vnb = pp.tile([S, B * D], BF16)
                nc.vector.tensor_copy(vnb[:], vnbf[:])

                # new-token scoresT for all batches: [4 keys, (b, h, s)]
                qT_v = qT_sb[:].rearrange("p (h t) -> p h t", h=HL)
                scn_ps = psT.tile([S, B * HL * S], F32, tag="scn")
                for b in range(B):
                    nc.tensor.matmul(
                        scn_ps[:, b * 16 : (b + 1) * 16],
                        kTn[:, S * b : S * b + S],
                        qT_v[:, :, S * b : S * b + S],
                        start=True,
                        stop=True,
                    )
                expn = pp.tile([S, B * HL * S], BF16)
                nc.scalar.activation(expn[:], scn_ps[:], Exp, scale=SCALE)
                # per-(b,h,s) sums of the 4 new-token weights: [1, 256]
                ns_ps = psT.tile([1, B * HL * S], F32, tag="nsum")
                nc.tensor.matmul(
                    ns_ps[:], onesb[0:S, 0:1], expn[:], start=True, stop=True
                )
                nsum = pp.tile([1, B * HL * S], F32)
                nc.scalar.copy(nsum[:], ns_ps[:])

            # ================= Phase B: attention over the KV cache =================
            outT_raw = pp.tile([128, HL * NTOK], BF16)  # cols h*64 + b*4 + s
            sums_all = pp.tile([1, HL * NTOK], F32)  # same column layout
            with (
                tc.tile_pool(name="kvf", bufs=2) as kvf,
                tc.tile_pool(name="kvb", bufs=2) as kvb,
                tc.tile_pool(name="exb", bufs=2) as exp_pool,
                tc.tile_pool(name="ps_sc", bufs=2, space="PSUM") as pssc,
                tc.tile_pool(name="ps_out", bufs=2, space="PSUM") as psout,
                tc.tile_pool(name="ps_sm", bufs=2, space="PSUM") as pssm,
            ):
                for b in range(B):
                    ktf = kvf.tile([128, TP], F32, tag="ktf")
                    nc.sync.dma_start(ktf[:], kT_d[b])
                    vtf = kvf.tile([128, TP], F32, tag="vtf")
                    nc.sync.dma_start(vtf[:], v_d[b])
                    # on-chip f32 -> bf16 (K on ScalarE, V on VectorE)
                    kt = kvb.tile([128, TP], BF16, tag="kt")
                    nc.scalar.copy(kt[:], ktf[:])
                    vt = kvb.tile([128, TP], BF16, tag="vt")
                    nc.vector.tensor_copy(vt[:], vtf[:])

                    qTb = qT_v[:, :, S * b : S * b + S]  # [128, 4, 4]

                    sc = pssc.tile([128, 512], F32, tag="sc")  # one PSUM bank
                    for c in range(NCH):
                        nc.tensor.matmul(
                            sc[:, c * 16 : (c + 1) * 16],
                            kt[:, c * 128 : (c + 1) * 128],
                            qTb,
                            start=True,
                            stop=True,
                        )

                    ex = exp_pool.tile([128, 512], BF16, tag="ex")
                    nc.scalar.activation(ex[:], sc[:], Exp, scale=SCALE)

                    outp = psout.tile([128, 16], F32, tag="outp")
                    for c in range(NCH):
                        nc.tensor.matmul(
                            outp[:],
                            vt[:, c * 128 : (c + 1) * 128],
                            ex[:, c * 16 : (c + 1) * 16],
                            start=(c == 0),
                            stop=False,
                        )
                    nc.tensor.matmul(
                        outp[:],
                        vnb[:, b * D : (b + 1) * D],
                        expn[:, b * 16 : (b + 1) * 16],
                        start=False,
                        stop=True,
                    )

                    # cache-key softmax partial sums: one [1,512] matmul + reduce
                    sm = pssm.tile([1, 512], F32, tag="sm")
                    nc.tensor.matmul(sm[:], onesb[:, 0:1], ex[:], start=True, stop=True)
                    smr = exp_pool.tile([1, 16], F32, tag="smr")
                    nc.vector.tensor_reduce(
                        smr[:],
                        sm[:].rearrange("o (c q) -> o q c", q=16),
                        mybir.AxisListType.X,
                        ADD,
                    )
                    # total sums = cache partials + new-token sums, scattered into
                    # the (h, b, s) column layout
                    nc.vector.scalar_tensor_tensor(
                        sums_all[:].rearrange("o (h bb t) -> o h bb t", h=HL, bb=B)[
                            :, :, b, :
                        ],
                        smr[:].rearrange("o (h t) -> o h t", h=HL),
                        1.0,
                        nsum[:, b * 16 : (b + 1) * 16].rearrange(
                            "o (h t) -> o h t", h=HL
                        ),
                        MULT,
                        ADD,
                    )
                    nc.scalar.copy(
                        outT_raw[:].rearrange("p (h bb t) -> p h bb t", h=HL, bb=B)[
                            :, :, b, :
                        ],
                        outp[:].rearrange("p (h t) -> p h t", h=HL),
                    )

            # ============ normalization (single pass over all batches) ============
            with (
                tc.tile_pool(name="nrm", bufs=1) as np_,
                tc.tile_pool(name="ps_bc", bufs=1, space="PSUM") as psbc,
            ):
                # subtract the NPAD pad keys' exp(0)=1 contributions, reciprocal
                sums_c = np_.tile([1, HL * NTOK], F32)
                nc.vector.tensor_scalar_add(sums_c[:], sums_all[:], float(-NPAD))
                rsf = np_.tile([1, HL * NTOK], F32)
                nc.vector.reciprocal(rsf[:], sums_c[:])
                bc = psbc.tile([128, HL * NTOK], F32)
                nc.tensor.matmul(bc[:], ones[0:1, :], rsf[:], start=True, stop=True)
                bcs = np_.tile([128, HL * NTOK], F32)
                nc.scalar.copy(bcs[:], bc[:])
                outT_all = pp.tile([128, HL * NTOK], BF16)
                nc.vector.scalar_tensor_tensor(
                    outT_all[:], outT_raw[:], 1.0, bcs[:], MULT, MULT
                )

            # ================= Phase C: o_proj partials =================
            with (
                tc.tile_pool(name="wC", bufs=3) as wc,
                tc.tile_pool(name="fin", bufs=2) as fp,
                tc.tile_pool(name="psC", bufs=2, space="PSUM") as psC,
            ):
                for n in range(HID // 512):
                    op_ps = psC.tile([NTOK, 512], F32, tag="op")
                    for h in range(HL):
                        wtf = wc.tile([128, 512], F32, tag="owf")
                        nc.sync.dma_start(
                            wtf[:], o_wT[h * 128 : (h + 1) * 128, n * 512 : (n + 1) * 512]
                        )
                        wt = wc.tile([128, 512], BF16, tag="ow")
                        nc.vector.tensor_copy(wt[:], wtf[:])
                        nc.tensor.matmul(
                            op_ps[:],
                            outT_all[:, h * NTOK : (h + 1) * NTOK],
                            wt[:],
                            start=(h == 0),
                            stop=(h == HL - 1),
                        )
                    fin = fp.tile([NTOK, 512], F32, tag="fin")
                    nc.scalar.copy(fin[:], op_ps[:])
                    nc.sync.dma_start(rs_in[:, n * 512 : (n + 1) * 512], fin[:])

            # ================= Phase D: ReduceScatter over batches =================
            nc.gpsimd.collective_compute(
                "ReduceScatter",
                mybir.AluOpType.add,
                ins=[rs_in.opt()],
                outs=[rs_out.opt()],
                replica_groups=[list(range(NCORES))],
            )
            nc.sync.dma_start(out_e[:], rs_out[:])

    nc.compile()
    return nc


_NC = None


def _get_nc():
    global _NC
    if _NC is None:
        _NC = build_graph()
    return _NC


def host_prep(
    hidden_states, positions, k_cache, v_cache, start_pos, q_w, q_b, k_w, k_b, v_w, v_b, o_w
):
    hs = np.ascontiguousarray(np.asarray(hidden_states, np.float32))
    pos = np.asarray(positions).reshape(NTOK)
    kc = np.asarray(k_cache, np.float32)
    vc = np.asarray(v_cache, np.float32)
    q_w = np.asarray(q_w, np.float32)
    q_b = np.asarray(q_b, np.float32)
    k_w = np.asarray(k_w, np.float32)
    k_b = np.asarray(k_b, np.float32)
    v_w = np.asarray(v_w, np.float32)
    v_b = np.asarray(v_b, np.float32)
    o_w = np.asarray(o_w, np.float32)
    assert int(start_pos) == TC, f"kernel compiled for start_pos={TC}"

    # [p, (c, t)] packing of hidden_states.T
    xTp = np.ascontiguousarray(
        hs.reshape(NTOK, 32, 128).transpose(2, 1, 0).reshape(128, 32 * NTOK)
    )
    half = D // 2
    inv_freq = (1.0 / (10000.0 ** (np.arange(0, half, dtype=np.float32) / half))).astype(
        np.float32
    )
    freqs = pos.astype(np.float32)[:, None] * inv_freq[None, :]
    cosN = np.cos(freqs).astype(np.float32)
    sinN = np.sin(freqs).astype(np.float32)

    maps = []
    for i in range(NCORES):
        kT = np.zeros((B, D, TP), np.float32)
        kT[:, :, :TC] = kc[:, i, :TC, :].transpose(0, 2, 1)
        vs = vc[:, i, :TC, :]  # [16, 4092, 128]
        vr = np.zeros((B, 128, NCH, 128), np.float32)
        vr[:, :, : NCH - 1] = vs[:, : (NCH - 1) * 128].reshape(
            B, NCH - 1, 128, 128
        ).transpose(0, 2, 1, 3)
        vr[:, : TC - (NCH - 1) * 128, NCH - 1] = vs[:, (NCH - 1) * 128 :]
        maps.append(
            dict(
                xTp=xTp,
                q_wT=np.ascontiguousarray(q_w[i * HDL : (i + 1) * HDL, :].T),
                q_b=np.ascontiguousarray(q_b[i * HDL : (i + 1) * HDL].reshape(1, HDL)),
                kv_wT=np.ascontiguousarray(
                    np.concatenate(
                        [k_w[i * D : (i + 1) * D], v_w[i * D : (i + 1) * D]], 0
                    ).T
                ),
                kv_b=np.ascontiguousarray(
                    np.concatenate(
                        [k_b[i * D : (i + 1) * D], v_b[i * D : (i + 1) * D]]
                    ).reshape(1, 2 * D)
                ),
                o_wT=np.ascontiguousarray(o_w[:, i * HDL : (i + 1) * HDL].T),
                kT=kT,
                v=vr.reshape(B, 128, TP),
                cosN=cosN,
                sinN=sinN,
            )
        )
    return maps


def kernel(**inputs):
    nc = _get_nc()
    in_maps = host_prep(**inputs)
    res = run_bass_kernel_spmd(nc, in_maps, list(range(NCORES))).results
    out = np.concatenate([res[i]["out"] for i in range(NCORES)], 0).reshape(B, S, HID)
    k_new = np.stack(
        [res[i]["k_new"].reshape(B, S, D) for i in range(NCORES)], axis=1
    )
    v_new = np.stack(
        [res[i]["v_new"].reshape(B, S, D) for i in range(NCORES)], axis=1
    )
    return out, k_new, v_new
